# revision 1
# baseline (speedup 1.0000x reference)
"""Trainium2 Bass kernel for nn_LinearMultiheadAttention (linear attention with
polynomial feature map phi(x) = [1, x, 0.5 x^2]), sharded over 8 NeuronCores.

Sharding: core c -> batch b = c//2, heads h0 = (c%2)*8 .. h0+8.
Each core computes a partial output (its 8 heads' contribution through Wo);
the host sums the two partials per batch.

Precision: the z = qsum*ksum normalizer is catastrophically ill-conditioned
(min |qsum| ~3e-4 while outputs reach 6e5), so q/k projections are computed
to full fp32 accuracy via an exact 3-term fp32r split
(hs = hi + lo, W = Whi + Wlo, 12+12 mantissa bits, products exact in the
fp32 PSUM accumulate; only the lo*lo term ~2^-24 is dropped). qsum/ksum are
accumulated in exact fp32. The v / kv / qkv / Wo path is bf16.
"""
import numpy as np
import ml_dtypes

import concourse.bass as bass
import concourse.tile as tile
from concourse import bacc, mybir
from concourse.bass_utils import run_bass_kernel_spmd

F32 = mybir.dt.float32
F32R = mybir.dt.float32r
BF16 = mybir.dt.bfloat16

B, S, D = 4, 4096, 1040
H, F, E = 16, 32, 65          # heads, feature_dim, head_dim (= 2F+1)
HPC = 8                        # heads per core
P = 128
NT = S // P                    # 32 token tiles per core
NCH = 9                        # ceil(D/128); last chunk K=16
KLAST = D - 8 * P              # 16
QW = HPC * F                   # 256 q (or k) cols per core
VW = HPC * E                   # 520 v cols per core
VH = 4 * E                     # 260
OCH = 5                        # ceil(VW/128); last chunk K=8
OLAST = VW - 4 * P             # 8

_CACHED = {}


def _chunk_k(c):
    return KLAST if c == NCH - 1 else P


def build_bass():
    nc = bacc.Bacc("TRN2", target_bir_lowering=False, debug=False, num_devices=8)
    hs = nc.dram_tensor("hs", [S, D], F32, kind="ExternalInput").ap()
    maskf = nc.dram_tensor("maskf", [P, NT], F32, kind="ExternalInput").ap()
    wqk = nc.dram_tensor("wqk", [NCH, P, 2 * QW], F32, kind="ExternalInput").ap()
    wv = nc.dram_tensor("wv", [NCH, P, VW], BF16, kind="ExternalInput").ap()
    wo = nc.dram_tensor("wo", [OCH, P, D], BF16, kind="ExternalInput").ap()
    id32 = nc.dram_tensor("id32", [P, P], F32, kind="ExternalInput").ap()
    id16 = nc.dram_tensor("id16", [P, P], BF16, kind="ExternalInput").ap()
    out = nc.dram_tensor("out", [S, D], F32, kind="ExternalOutput").ap()

    with tile.TileContext(nc) as tc:
        with (
            tc.tile_pool(name="consts", bufs=1) as consts,
            tc.tile_pool(name="state", bufs=1) as state,
            tc.tile_pool(name="rot", bufs=2) as rot,
            tc.tile_pool(name="rot3", bufs=3) as rot3,
            tc.tile_pool(name="ps", bufs=1, space="PSUM") as ps,
            tc.tile_pool(name="ps2", bufs=2, space="PSUM") as ps2,
        ):
            # ---- constants ----
            wqk_sb = consts.tile([P, NCH, 2 * QW], F32)
            nc.sync.dma_start(out=wqk_sb, in_=wqk.rearrange("c p j -> p c j"))
            wv_sb = consts.tile([P, NCH, VW], BF16)
            nc.sync.dma_start(out=wv_sb, in_=wv.rearrange("c p j -> p c j"))
            wo_sb = consts.tile([P, OCH, D], BF16)
            nc.sync.dma_start(out=wo_sb, in_=wo.rearrange("c p j -> p c j"))
            id32_sb = consts.tile([P, P], F32)
            nc.sync.dma_start(out=id32_sb, in_=id32)
            id16_sb = consts.tile([P, P], BF16)
            nc.sync.dma_start(out=id16_sb, in_=id16)
            mask_sb = consts.tile([P, NT], F32)
            nc.sync.dma_start(out=mask_sb, in_=maskf)
            ones_col = consts.tile([P, 1], F32)
            nc.vector.memset(ones_col[:], 1.0)
            half_col = consts.tile([P, 1], F32)
            nc.vector.memset(half_col[:], 0.5)
            # fp32r hi/lo split of [Wq|Wk] (exact: 12+12 mantissa bits)
            wqkr = consts.tile([P, NCH, 2 * QW], F32R)
            nc.vector.tensor_copy(wqkr[:], wqk_sb[:])
            wqkl = consts.tile([P, NCH, 2 * QW], F32R)
            nc.vector.tensor_tensor(wqkl[:], wqk_sb[:], wqkr[:].bitcast(F32),
                                    mybir.AluOpType.subtract)

            # ---- persistent state ----
            phiq = state.tile([P, NT, HPC, E], BF16)   # rq-folded phi(q) stash
            kvs_sb = state.tile([E, HPC * E], BF16)    # rksum-scaled kv
            rk_row = state.tile([1, HPC * E], F32)
            rk_bc = state.tile([P, HPC * E], F32)
            klin_acc = state.tile([P, QW], F32)
            ksq_acc = state.tile([P, QW], F32)
            cs_sb = state.tile([P, 4], F32)
            kt4 = state.tile([4, P], F32)
            kv_ps = [ps.tile([E, VH], F32, tag=f"kv{i}", name=f"kv{i}")
                     for i in range(2)]

            # =============== PASS A ===============
            for t in range(NT):
                with nc.named_scope(f"A{t}"):
                    hs_t = rot3.tile([P, D], F32, tag="hs")
                    nc.sync.dma_start(out=hs_t, in_=hs[t * P:(t + 1) * P, :])

                    hsr = rot.tile([P, NCH, P], F32R, tag="hsr")
                    hlo = rot.tile([P, NCH, P], F32R, tag="hlo")
                    hsT16 = rot.tile([P, NCH, P], BF16, tag="hsT16")
                    for g, cs in enumerate([range(0, 4), range(4, 8), range(8, 9)]):
                        tp = ps2.tile([P, 512], F32, tag="tps",
                                      name=f"tp_{t}_{g}")
                        for c in cs:
                            kk = _chunk_k(c)
                            nc.tensor.transpose(
                                tp[0:kk, (c % 4) * P:(c % 4) * P + P],
                                hs_t[:, c * P:c * P + kk],
                                id32_sb[:],
                            )
                        lo, hi = cs[0], cs[-1] + 1
                        kk = _chunk_k(hi - 1)
                        w = (hi - 1 - lo) * P + P
                        src = tp[0:kk, 0:w]
                        hr = hsr[0:kk, lo:hi, :].rearrange("p c n -> p (c n)")
                        nc.scalar.activation(hr, src,
                                             mybir.ActivationFunctionType.Copy)
                        nc.vector.tensor_tensor(
                            hlo[0:kk, lo:hi, :].rearrange("p c n -> p (c n)"),
                            src, hr.bitcast(F32), mybir.AluOpType.subtract)
                        nc.scalar.activation(
                            hsT16[0:kk, lo:hi, :].rearrange("p c n -> p (c n)"), src,
                            mybir.ActivationFunctionType.Copy)

                    # projections: q|k via exact 3-term fp32r, v via bf16
                    qk_ps = ps2.tile([P, 2 * QW], F32, tag="qk", name=f"qk_{t}")
                    v1_ps = ps.tile([P, VH], F32, tag="v1", name=f"v1_{t}")
                    v2_ps = ps.tile([P, VH], F32, tag="v2", name=f"v2_{t}")
                    for c in range(NCH):
                        kk = _chunk_k(c)
                        nc.tensor.matmul(
                            qk_ps[:], hsr[0:kk, c, :], wqkr[0:kk, c, :],
                            start=(c == 0), stop=False, skip_group_check=True)
                        nc.tensor.matmul(
                            qk_ps[:], hsr[0:kk, c, :], wqkl[0:kk, c, :],
                            start=False, stop=False, skip_group_check=True)
                        nc.tensor.matmul(
                            v1_ps[:], hsT16[0:kk, c, :], wv_sb[0:kk, c, 0:VH],
                            start=(c == 0), stop=(c == NCH - 1))
                        nc.tensor.matmul(
                            v2_ps[:], hsT16[0:kk, c, :], wv_sb[0:kk, c, VH:VW],
                            start=(c == 0), stop=(c == NCH - 1))
                    for c in range(NCH):
                        kk = _chunk_k(c)
                        nc.tensor.matmul(
                            qk_ps[:], hlo[0:kk, c, :], wqkr[0:kk, c, :],
                            start=False, stop=(c == NCH - 1),
                            skip_group_check=True)

                    # exact fp32 copies + squares
                    qf32 = rot.tile([P, QW], F32, tag="qf32")
                    nc.scalar.activation(qf32[:], qk_ps[:, 0:QW],
                                         mybir.ActivationFunctionType.Copy)
                    kf32 = rot.tile([P, QW], F32, tag="kf32")
                    nc.scalar.activation(kf32[:], qk_ps[:, QW:2 * QW],
                                         mybir.ActivationFunctionType.Copy)
                    sq2 = rot.tile([P, QW], F32, tag="sq2")
                    nc.vector.tensor_mul(sq2[:], qk_ps[:, 0:QW], qf32[:])
                    sk2 = rot.tile([P, QW], F32, tag="sk2")
                    nc.vector.tensor_mul(sk2[:], qk_ps[:, QW:2 * QW], kf32[:])

                    # ksum accumulators (per-partition partial sums, exact fp32)
                    if t == 0:
                        nc.vector.tensor_copy(klin_acc[:], kf32[:])
                        nc.vector.tensor_copy(ksq_acc[:], sk2[:])
                    else:
                        nc.vector.tensor_add(klin_acc[:], klin_acc[:], kf32[:])
                        nc.vector.tensor_add(ksq_acc[:], ksq_acc[:], sk2[:])

                    # qsum = 1 + sum(q) + 0.5*sum(q^2); rq = mask/qsum
                    sumq = rot.tile([P, HPC], F32, tag="sumq")
                    nc.vector.tensor_reduce(
                        sumq[:], qf32[:].rearrange("p (h f) -> p h f", f=F),
                        mybir.AxisListType.X, mybir.AluOpType.add)
                    sumq2 = rot.tile([P, HPC], F32, tag="sumq2")
                    nc.vector.tensor_reduce(
                        sumq2[:], sq2[:].rearrange("p (h f) -> p h f", f=F),
                        mybir.AxisListType.X, mybir.AluOpType.add)
                    qsum = rot.tile([P, HPC], F32, tag="qsum")
                    nc.vector.tensor_scalar(
                        qsum[:], sumq2[:], 0.5, 1.0,
                        mybir.AluOpType.mult, mybir.AluOpType.add)
                    nc.vector.tensor_add(qsum[:], qsum[:], sumq[:])
                    rq = rot.tile([P, HPC], F32, tag="rq")
                    nc.vector.reciprocal(rq[:], qsum[:])
                    nc.vector.tensor_mul(
                        rq[:], rq[:], mask_sb[:, t:t + 1].broadcast_to([P, HPC]))
                    rq05 = rot.tile([P, HPC], F32, tag="rq05")
                    nc.vector.tensor_scalar_mul(rq05[:], rq[:], 0.5)

                    # phi_q (rq folded) -> stash (bf16); gpsimd takes sbuf-only ops
                    pq = phiq[:, t]                      # [P, HPC, E]
                    nc.gpsimd.tensor_copy(pq[:, :, 0:1], rq[:].unsqueeze(2))
                    nc.vector.tensor_mul(
                        pq[:, :, 1:1 + F],
                        qf32[:].rearrange("p (h f) -> p h f", f=F),
                        rq[:].unsqueeze(2).broadcast_to([P, HPC, F]))
                    nc.gpsimd.tensor_mul(
                        pq[:, :, 1 + F:E],
                        sq2[:].rearrange("p (h f) -> p h f", f=F),
                        rq05[:].unsqueeze(2).broadcast_to([P, HPC, F]))

                    # phi_k (bf16) and v (bf16)
                    phik = rot.tile([P, HPC, E], BF16, tag="phik")
                    nc.gpsimd.memset(phik[:, :, 0:1], 1.0)
                    nc.gpsimd.tensor_copy(
                        phik[:, :, 1:1 + F],
                        kf32[:].rearrange("p (h f) -> p h f", f=F))
                    nc.gpsimd.tensor_scalar_mul(
                        phik[:, :, 1 + F:E],
                        sk2[:].rearrange("p (h f) -> p h f", f=F), 0.5)
                    v16 = rot.tile([P, VW], BF16, tag="v16")
                    nc.scalar.activation(v16[:, 0:VH], v1_ps[:],
                                         mybir.ActivationFunctionType.Copy)
                    nc.scalar.activation(v16[:, VH:VW], v2_ps[:],
                                         mybir.ActivationFunctionType.Copy)

                    # kv per head -> persistent psum accumulators
                    for h in range(HPC):
                        nc.tensor.matmul(
                            kv_ps[h // 4][:, (h % 4) * E:(h % 4) * E + E],
                            phik[:, h, :], v16[:, h * E:h * E + E],
                            start=(t == 0 and h % 4 == 0), stop=(t == NT - 1),
                            skip_group_check=True)

            # =============== MID: ksum assembly ===============
            with nc.named_scope("mid"):
                cs_ps = ps.tile([P, 4], F32, tag="v2", name="cs_ps")
                for j in range(2):
                    nc.tensor.matmul(
                        cs_ps[:, j:j + 1], klin_acc[:, j * P:(j + 1) * P],
                        ones_col[:], start=(j == 0), stop=False,
                        skip_group_check=True)
                    nc.tensor.matmul(
                        cs_ps[:, 2 + j:3 + j], ksq_acc[:, j * P:(j + 1) * P],
                        half_col[:], start=False, stop=(j == 1),
                        skip_group_check=True)
                nc.vector.tensor_copy(cs_sb[:], cs_ps[:])
                csT_ps = ps.tile([4, P], F32, tag="v1", name="csT")
                nc.tensor.transpose(csT_ps[:], cs_sb[:], id32_sb[:])
                nc.vector.tensor_copy(kt4[:], csT_ps[:])

                rk_view = rk_row[:].rearrange("o (h e) -> o h e", e=E)
                nc.vector.memset(rk_view[:, :, 0:1], float(S))
                # kt4 rows: 0 = sum(k) cols 0:128, 1 = cols 128:256,
                #           2 = 0.5*sum(k^2) 0:128, 3 = 128:256.
                # Row 0 is on partition 0 (DVE); rows 1-3 need partition
                # shifts -> small SBUF->SBUF DMAs.
                nc.vector.tensor_copy(
                    rk_view[:, 0:4, 1:1 + F],
                    kt4[0:1, :].rearrange("o (h f) -> o h f", f=F))
                nc.sync.dma_start(
                    out=rk_view[:, 4:8, 1:1 + F],
                    in_=kt4[1:2, :].rearrange("o (h f) -> o h f", f=F))
                nc.sync.dma_start(
                    out=rk_view[:, 0:4, 1 + F:E],
                    in_=kt4[2:3, :].rearrange("o (h f) -> o h f", f=F))
                nc.sync.dma_start(
                    out=rk_view[:, 4:8, 1 + F:E],
                    in_=kt4[3:4, :].rearrange("o (h f) -> o h f", f=F))
                nc.vector.reciprocal(rk_row[:], rk_row[:])
                nc.gpsimd.partition_broadcast(rk_bc[:], rk_row[:])

                for i in range(2):
                    nc.vector.tensor_mul(
                        kvs_sb[:, i * VH:(i + 1) * VH],
                        kv_ps[i][:],
                        rk_bc[0:E, i * VH:(i + 1) * VH])

            # =============== PASS B ===============
            for t in range(NT):
                with nc.named_scope(f"B{t}"):
                    phiT_sb = rot.tile([E, HPC, P], BF16, tag="phiT")
                    for g in range(2):
                        tp = ps2.tile([P, 512], BF16, tag="tps",
                                      name=f"ptp_{t}_{g}")
                        for hh in range(4):
                            h = g * 4 + hh
                            nc.tensor.transpose(
                                tp[0:E, hh * P:hh * P + P],
                                phiq[:, t, h, :], id16_sb[:])
                        nc.vector.tensor_copy(
                            phiT_sb[:, g * 4:(g + 1) * 4, :].rearrange(
                                "p h n -> p (h n)"),
                            tp[0:E, :])

                    o_ps = [ps.tile([P, VH], F32, tag="v1", name=f"ops0_{t}"),
                            ps.tile([P, VH], F32, tag="v2", name=f"ops1_{t}")]
                    for h in range(HPC):
                        nc.tensor.matmul(
                            o_ps[h // 4][:, (h % 4) * E:(h % 4) * E + E],
                            phiT_sb[:, h, :], kvs_sb[:, h * E:h * E + E],
                            start=(h % 4 == 0), stop=(h % 4 == 3),
                            skip_group_check=True)
                    o_sb = rot.tile([P, VW], BF16, tag="osb")
                    nc.vector.tensor_copy(o_sb[:, 0:VH], o_ps[0][:])
                    nc.scalar.activation(o_sb[:, VH:VW], o_ps[1][:],
                                         mybir.ActivationFunctionType.Copy)

                    oT_sb = rot.tile([P, OCH, P], BF16, tag="oT")
                    for g, cs in enumerate([range(0, 4), range(4, 5)]):
                        tp = ps2.tile([P, 512], BF16, tag="tps",
                                      name=f"otp_{t}_{g}")
                        for c in cs:
                            kk = OLAST if c == OCH - 1 else P
                            nc.tensor.transpose(
                                tp[0:kk, (c % 4) * P:(c % 4) * P + P],
                                o_sb[:, c * P:c * P + kk], id16_sb[:])
                        lo, hi = cs[0], cs[-1] + 1
                        kk = OLAST if hi == OCH else P
                        w = (hi - 1 - lo) * P + P
                        nc.vector.tensor_copy(
                            oT_sb[0:kk, lo:hi, :].rearrange("p c n -> p (c n)"),
                            tp[0:kk, 0:w])

                    f1 = ps2.tile([P, 512], F32, tag="qk", name=f"f1_{t}")
                    f2 = ps2.tile([P, 512], F32, tag="qk", name=f"f2_{t}")
                    f3 = ps.tile([P, D - 1024], F32, tag="v1", name=f"f3_{t}")
                    for c in range(OCH):
                        kk = OLAST if c == OCH - 1 else P
                        nc.tensor.matmul(f1[:], oT_sb[0:kk, c, :],
                                         wo_sb[0:kk, c, 0:512],
                                         start=(c == 0), stop=(c == OCH - 1))
                        nc.tensor.matmul(f2[:], oT_sb[0:kk, c, :],
                                         wo_sb[0:kk, c, 512:1024],
                                         start=(c == 0), stop=(c == OCH - 1))
                        nc.tensor.matmul(f3[:], oT_sb[0:kk, c, :],
                                         wo_sb[0:kk, c, 1024:D],
                                         start=(c == 0), stop=(c == OCH - 1))
                    out_sb = rot.tile([P, D], F32, tag="outsb")
                    nc.vector.tensor_copy(out_sb[:, 0:512], f1[:])
                    nc.scalar.activation(out_sb[:, 512:1024], f2[:],
                                         mybir.ActivationFunctionType.Copy)
                    nc.vector.tensor_copy(out_sb[:, 1024:D], f3[:])
                    nc.sync.dma_start(out=out[t * P:(t + 1) * P, :], in_=out_sb)

    nc.compile()
    return nc


def _prep_core_inputs(hidden_states, attention_mask, Wq, Wk, Wv, Wo, core):
    b, half = core // 2, core % 2
    h0 = half * HPC
    bf = ml_dtypes.bfloat16

    hs = np.ascontiguousarray(hidden_states[b]).astype(np.float32)
    maskf = np.ascontiguousarray(
        attention_mask[b].astype(np.float32).reshape(NT, P).T)

    def chunks(w):
        out = np.zeros((NCH, P, w.shape[1]), dtype=np.float32)
        for c in range(NCH):
            kk = _chunk_k(c)
            out[c, 0:kk] = w[c * P:c * P + kk]
        return out

    wq_h = Wq[:, h0 * F:(h0 + HPC) * F].astype(np.float32)
    wk_h = Wk[:, h0 * F:(h0 + HPC) * F].astype(np.float32)
    wqk_h = chunks(np.concatenate([wq_h, wk_h], axis=1))
    wv_h = chunks(Wv[:, h0 * E:(h0 + HPC) * E].astype(np.float32)).astype(bf)
    wo_rows = Wo[h0 * E:(h0 + HPC) * E].astype(np.float32)
    wo_h = np.zeros((OCH, P, D), dtype=np.float32)
    for c in range(OCH):
        kk = OLAST if c == OCH - 1 else P
        wo_h[c, 0:kk] = wo_rows[c * P:c * P + kk]
    wo_h = wo_h.astype(bf)

    return {
        "hs": hs,
        "maskf": maskf,
        "wqk": wqk_h,
        "wv": wv_h,
        "wo": wo_h,
        "id32": np.eye(P, dtype=np.float32),
        "id16": np.eye(P, dtype=np.float32).astype(bf),
    }


def kernel(hidden_states, attention_mask, Wq, Wk, Wv, Wo, _trace=False):
    hidden_states = np.asarray(hidden_states)
    attention_mask = np.asarray(attention_mask)
    Wq = np.asarray(Wq); Wk = np.asarray(Wk)
    Wv = np.asarray(Wv); Wo = np.asarray(Wo)

    if "nc" not in _CACHED:
        _CACHED["nc"] = build_bass()
    nc = _CACHED["nc"]

    in_maps = [
        _prep_core_inputs(hidden_states, attention_mask, Wq, Wk, Wv, Wo, c)
        for c in range(8)
    ]
    res = run_bass_kernel_spmd(nc, in_maps, core_ids=list(range(8)),
                               trace=_trace)
    _CACHED["last_result"] = res
    out = np.empty((B, S, D), dtype=np.float32)
    for b in range(B):
        out[b] = res.results[2 * b]["out"] + res.results[2 * b + 1]["out"]
    return out



# revision 9
# speedup vs baseline: 1.5636x; 1.5636x over previous
"""Trainium2 Bass kernel for nn_LinearMultiheadAttention (linear attention with
polynomial feature map phi(x) = [1, x, 0.5 x^2]), sharded over 8 NeuronCores.

Sharding: core c -> batch b = c//2, heads h0 = (c%2)*8 .. h0+8.
Each core computes a partial output (its 8 heads' contribution through Wo);
the host sums the two partials per batch.

Precision: the z = qsum*ksum normalizer is catastrophically ill-conditioned
(min |qsum| ~3e-4 while outputs reach 6e5), so q/k projections are computed
to full fp32 accuracy via an exact 3-term fp32r split
(hs = hi + lo, W = Whi + Wlo, 12+12 mantissa bits, products exact in the
fp32 PSUM accumulate; only the lo*lo term ~2^-24 is dropped). qsum/ksum are
accumulated in exact fp32. The v / kv / qkv / Wo path is bf16.

v2 scheduling: kv matmuls deferred one tile (no PE head-of-line block on the
phi_k build), no gpsimd anywhere (measured ~20x below spec), matmul-based
ksum assembly in mid (no SBUF-SBUF DMAs / transposes), hs prefetched before
the big weight DMAs, pass B software-pipelined 3 deep with single-bank bf16
transpose targets.
"""
import numpy as np
import ml_dtypes

import concourse.bass as bass
import concourse.tile as tile
from concourse import bacc, mybir
from concourse.bass_utils import run_bass_kernel_spmd

F32 = mybir.dt.float32
F32R = mybir.dt.float32r
BF16 = mybir.dt.bfloat16

B, S, D = 4, 4096, 1040
H, F, E = 16, 32, 65          # heads, feature_dim, head_dim (= 2F+1)
HPC = 8                        # heads per core
P = 128
NT = S // P                    # 32 token tiles per core
NCH = 9                        # ceil(D/128); last chunk K=16
KLAST = D - 8 * P              # 16
QW = HPC * F                   # 256 q (or k) cols per core
VW = HPC * E                   # 520 v cols per core
VH = 4 * E                     # 260
OCH = 5                        # ceil(VW/128); last chunk K=8
OLAST = VW - 4 * P             # 8

_CACHED = {}


def _chunk_k(c):
    return KLAST if c == NCH - 1 else P


def build_bass():
    nc = bacc.Bacc("TRN2", target_bir_lowering=False, debug=False, num_devices=8)
    hs = nc.dram_tensor("hs", [S, D], F32, kind="ExternalInput").ap()
    maskf = nc.dram_tensor("maskf", [P, NT], F32, kind="ExternalInput").ap()
    wqk = nc.dram_tensor("wqk", [NCH, P, 2 * QW], F32, kind="ExternalInput").ap()
    wv = nc.dram_tensor("wv", [NCH, P, VW], BF16, kind="ExternalInput").ap()
    wo = nc.dram_tensor("wo", [OCH, P, D], BF16, kind="ExternalInput").ap()
    id32 = nc.dram_tensor("id32", [P, P], F32, kind="ExternalInput").ap()
    id16 = nc.dram_tensor("id16", [P, P], BF16, kind="ExternalInput").ap()
    out = nc.dram_tensor("out", [S, D], F32, kind="ExternalOutput").ap()

    ACT = mybir.ActivationFunctionType.Copy

    with tile.TileContext(nc) as tc:
        with (
            tc.tile_pool(name="consts", bufs=1) as consts,
            tc.tile_pool(name="state", bufs=1) as state,
            tc.tile_pool(name="rot", bufs=2) as rot,
            tc.tile_pool(name="ps", bufs=1, space="PSUM") as ps,
            tc.tile_pool(name="ps2", bufs=2, space="PSUM") as ps2,
        ):
            # ---- small consts + hs prefetch BEFORE the big weight DMAs ----
            id32_sb = consts.tile([P, P], F32)
            nc.sync.dma_start(out=id32_sb, in_=id32)
            id16_sb = consts.tile([P, P], BF16)
            nc.sync.dma_start(out=id16_sb, in_=id16)
            mask_sb = consts.tile([P, NT], F32)
            nc.sync.dma_start(out=mask_sb, in_=maskf)

            hs_tiles = [consts.tile([P, D], F32, name=f"hst{i}")
                        for i in range(3)]
            for i in range(3):
                nc.sync.dma_start(out=hs_tiles[i], in_=hs[i * P:(i + 1) * P, :])

            wqk_sb = consts.tile([P, NCH, 2 * QW], F32)
            nc.sync.dma_start(out=wqk_sb, in_=wqk.rearrange("c p j -> p c j"))
            wv_sb = consts.tile([P, NCH, VW], BF16)
            nc.sync.dma_start(out=wv_sb, in_=wv.rearrange("c p j -> p c j"))
            wo_sb = consts.tile([P, OCH, D], BF16)
            nc.sync.dma_start(out=wo_sb, in_=wo.rearrange("c p j -> p c j"))

            ones_col = consts.tile([P, 1], F32)
            nc.vector.memset(ones_col[:], 1.0)
            half_col = consts.tile([P, 1], F32)
            nc.vector.memset(half_col[:], 0.5)
            ones_row = consts.tile([1, P], F32)
            nc.vector.memset(ones_row[:], 1.0)
            # fp32r hi/lo split of [Wq|Wk] (exact: 12+12 mantissa bits)
            wqkr = consts.tile([P, NCH, 2 * QW], F32R)
            nc.vector.tensor_copy(wqkr[:], wqk_sb[:])
            wqkl = consts.tile([P, NCH, 2 * QW], F32R)
            nc.vector.tensor_tensor(wqkl[:], wqk_sb[:], wqkr[:].bitcast(F32),
                                    mybir.AluOpType.subtract)

            # ---- persistent state ----
            phiq = state.tile([P, NT, HPC, E], BF16)   # rq-folded phi(q) stash
            kvs_sb = state.tile([E, HPC * E], BF16)    # rksum-scaled kv
            rk_row = state.tile([1, HPC * E], F32)
            klin_acc = state.tile([P, QW], F32)
            ksq_acc = state.tile([P, QW], F32)
            kv_ps = [ps.tile([E, VH], F32, tag=f"kv{i}", name=f"kv{i}")
                     for i in range(2)]
            # phik/v16 live one extra tile (kv deferral) -> explicit 2 bufs
            phik_t = [state.tile([P, HPC, E], BF16, name=f"phik{i}")
                      for i in range(2)]
            v16_t = [state.tile([P, VW], BF16, name=f"v16{i}")
                     for i in range(2)]
            # ones column of phi_k never changes: set once per buffer
            for i in range(2):
                nc.vector.memset(phik_t[i][:, :, 0:1], 1.0)

            def kv_mm(t):
                pk, vv = phik_t[t % 2], v16_t[t % 2]
                for h in range(HPC):
                    nc.tensor.matmul(
                        kv_ps[h // 4][:, (h % 4) * E:(h % 4) * E + E],
                        pk[:, h, :], vv[:, h * E:h * E + E],
                        start=(t == 0 and h % 4 == 0), stop=(t == NT - 1),
                        skip_group_check=True)

            # =============== PASS A ===============
            for t in range(NT):
                with nc.named_scope(f"A{t}"):
                    hs_t = hs_tiles[t % 3]

                    hsr = rot.tile([P, NCH, P], F32R, tag="hsr")
                    hlo = rot.tile([P, NCH, P], F32R, tag="hlo")
                    hsT16 = rot.tile([P, NCH, P], BF16, tag="hsT16")
                    for g, cs in enumerate([range(0, 4), range(4, 8), range(8, 9)]):
                        tp = ps2.tile([P, 512], F32, tag="tps",
                                      name=f"tp_{t}_{g}")
                        for c in cs:
                            kk = _chunk_k(c)
                            nc.tensor.transpose(
                                tp[0:kk, (c % 4) * P:(c % 4) * P + P],
                                hs_t[:, c * P:c * P + kk],
                                id32_sb[:])
                        lo, hi = cs[0], cs[-1] + 1
                        kk = _chunk_k(hi - 1)
                        w = (hi - 1 - lo) * P + P
                        src = tp[0:kk, 0:w]
                        hr = hsr[0:kk, lo:hi, :].rearrange("p c n -> p (c n)")
                        nc.scalar.activation(hr, src, ACT)
                        nc.vector.tensor_tensor(
                            hlo[0:kk, lo:hi, :].rearrange("p c n -> p (c n)"),
                            src, hr.bitcast(F32), mybir.AluOpType.subtract)
                        nc.scalar.activation(
                            hsT16[0:kk, lo:hi, :].rearrange("p c n -> p (c n)"),
                            src, ACT)

                    # refill this hs slot for tile t+3 (readers above queued)
                    if t + 3 < NT:
                        nc.sync.dma_start(
                            out=hs_t, in_=hs[(t + 3) * P:(t + 4) * P, :])

                    # projections: q|k via exact 3-term fp32r, v via bf16
                    qk_ps = ps2.tile([P, 2 * QW], F32, tag="qk", name=f"qk_{t}")
                    v1_ps = ps.tile([P, VH], F32, tag="v1", name=f"v1_{t}")
                    v2_ps = ps.tile([P, VH], F32, tag="v2", name=f"v2_{t}")
                    for c in range(NCH):
                        kk = _chunk_k(c)
                        nc.tensor.matmul(
                            qk_ps[:], hsr[0:kk, c, :], wqkr[0:kk, c, :],
                            start=(c == 0), stop=False, skip_group_check=True)
                        nc.tensor.matmul(
                            qk_ps[:], hsr[0:kk, c, :], wqkl[0:kk, c, :],
                            start=False, stop=False, skip_group_check=True)
                        nc.tensor.matmul(
                            v1_ps[:], hsT16[0:kk, c, :], wv_sb[0:kk, c, 0:VH],
                            start=(c == 0), stop=(c == NCH - 1))
                        nc.tensor.matmul(
                            v2_ps[:], hsT16[0:kk, c, :], wv_sb[0:kk, c, VH:VW],
                            start=(c == 0), stop=(c == NCH - 1))
                    for c in range(NCH):
                        kk = _chunk_k(c)
                        nc.tensor.matmul(
                            qk_ps[:], hlo[0:kk, c, :], wqkr[0:kk, c, :],
                            start=False, stop=(c == NCH - 1),
                            skip_group_check=True)

                    # kv for the PREVIOUS tile (its phik/v16 are long ready,
                    # so the PE never blocks on the vector chain below)
                    if t > 0:
                        kv_mm(t - 1)

                    # exact fp32 copies + squares
                    qf32 = rot.tile([P, QW], F32, tag="qf32")
                    nc.scalar.activation(qf32[:], qk_ps[:, 0:QW], ACT)
                    kf32 = rot.tile([P, QW], F32, tag="kf32")
                    nc.scalar.activation(kf32[:], qk_ps[:, QW:2 * QW], ACT)
                    sq2 = rot.tile([P, QW], F32, tag="sq2")
                    nc.vector.tensor_mul(sq2[:], qk_ps[:, 0:QW], qf32[:])
                    sk2 = rot.tile([P, QW], F32, tag="sk2")
                    nc.vector.tensor_mul(sk2[:], qk_ps[:, QW:2 * QW], kf32[:])

                    # ksum accumulators (per-partition partial sums, exact fp32)
                    if t == 0:
                        nc.vector.tensor_copy(klin_acc[:], kf32[:])
                        nc.vector.tensor_copy(ksq_acc[:], sk2[:])
                    else:
                        nc.vector.tensor_add(klin_acc[:], klin_acc[:], kf32[:])
                        nc.vector.tensor_add(ksq_acc[:], ksq_acc[:], sk2[:])

                    # qsum = 1 + sum(q) + 0.5*sum(q^2); rq = mask/qsum
                    sumq = rot.tile([P, HPC], F32, tag="sumq")
                    nc.vector.tensor_reduce(
                        sumq[:], qf32[:].rearrange("p (h f) -> p h f", f=F),
                        mybir.AxisListType.X, mybir.AluOpType.add)
                    sumq2 = rot.tile([P, HPC], F32, tag="sumq2")
                    nc.vector.tensor_reduce(
                        sumq2[:], sq2[:].rearrange("p (h f) -> p h f", f=F),
                        mybir.AxisListType.X, mybir.AluOpType.add)
                    qsum = rot.tile([P, HPC], F32, tag="qsum")
                    nc.vector.tensor_scalar(
                        qsum[:], sumq2[:], 0.5, 1.0,
                        mybir.AluOpType.mult, mybir.AluOpType.add)
                    nc.vector.tensor_add(qsum[:], qsum[:], sumq[:])
                    rq = rot.tile([P, HPC], F32, tag="rq")
                    nc.vector.reciprocal(rq[:], qsum[:])
                    nc.vector.tensor_mul(
                        rq[:], rq[:], mask_sb[:, t:t + 1].broadcast_to([P, HPC]))
                    rq05 = rot.tile([P, HPC], F32, tag="rq05")
                    nc.vector.tensor_scalar_mul(rq05[:], rq[:], 0.5)

                    # phi_q (rq folded) -> stash (bf16)
                    pq = phiq[:, t]                      # [P, HPC, E]
                    nc.vector.tensor_copy(pq[:, :, 0:1], rq[:].unsqueeze(2))
                    nc.vector.tensor_mul(
                        pq[:, :, 1:1 + F],
                        qf32[:].rearrange("p (h f) -> p h f", f=F),
                        rq[:].unsqueeze(2).broadcast_to([P, HPC, F]))
                    nc.vector.tensor_mul(
                        pq[:, :, 1 + F:E],
                        sq2[:].rearrange("p (h f) -> p h f", f=F),
                        rq05[:].unsqueeze(2).broadcast_to([P, HPC, F]))

                    # phi_k (bf16, ones col preset) and v (bf16)
                    pk = phik_t[t % 2]
                    nc.scalar.activation(
                        pk[:, :, 1:1 + F],
                        kf32[:].rearrange("p (h f) -> p h f", f=F), ACT)
                    nc.vector.tensor_scalar_mul(
                        pk[:, :, 1 + F:E],
                        sk2[:].rearrange("p (h f) -> p h f", f=F), 0.5)
                    v16 = v16_t[t % 2]
                    nc.scalar.activation(v16[:, 0:VH], v1_ps[:], ACT)
                    nc.scalar.activation(v16[:, VH:VW], v2_ps[:], ACT)

            kv_mm(NT - 1)

            # =============== MID: ksum assembly (matmul-based) ===============
            with nc.named_scope("mid"):
                # column sums of klin/ksq -> [1, 512] on partition 0
                sums_ps = ps2.tile([1, 512], F32, tag="tps", name="sums_ps")
                nc.tensor.matmul(sums_ps[:, 0:QW], ones_col[:], klin_acc[:],
                                 start=True, stop=False, skip_group_check=True)
                nc.tensor.matmul(sums_ps[:, QW:2 * QW], half_col[:], ksq_acc[:],
                                 start=True, stop=True, skip_group_check=True)

                rk_view = rk_row[:].rearrange("o (h e) -> o h e", e=E)
                nc.vector.memset(rk_view[:, :, 0:1], float(S))
                nc.vector.tensor_copy(
                    rk_view[:, :, 1:1 + F],
                    sums_ps[:, 0:QW].rearrange("o (h f) -> o h f", f=F))
                nc.vector.tensor_copy(
                    rk_view[:, :, 1 + F:E],
                    sums_ps[:, QW:2 * QW].rearrange("o (h f) -> o h f", f=F))
                nc.vector.reciprocal(rk_row[:], rk_row[:])

                # broadcast rk over 65 partitions via PE, then scale kv
                rk_sb = state.tile([E, HPC * E], F32)
                for i in range(2):
                    rk_ps = ps.tile([E, VH], F32, tag=f"v{i + 1}",
                                    name=f"rk_ps{i}")
                    nc.tensor.matmul(rk_ps[:], ones_row[:, 0:E],
                                     rk_row[:, i * VH:(i + 1) * VH],
                                     start=True, stop=True,
                                     skip_group_check=True)
                    nc.scalar.activation(rk_sb[:, i * VH:(i + 1) * VH],
                                         rk_ps[:], ACT)
                    nc.vector.tensor_mul(
                        kvs_sb[:, i * VH:(i + 1) * VH],
                        kv_ps[i][:], rk_sb[:, i * VH:(i + 1) * VH])

            # =============== PASS B (3-deep software pipeline) ===============
            # stages for tile t: T=phiq transpose, O=o matmuls, R=o transpose,
            # W=wo matmuls. body(t) issues T(t+1) O(t) R(t-1) W(t-2).
            phiT_sbs, o_sbs, oT_sbs = {}, {}, {}

            def stage_T(t):                      # phiq -> phiT_sb [E, HPC, P]
                tp = ps2.tile([E, HPC, P], BF16, tag="tps", name=f"ptp_{t}",
                              bufs=2)
                for h in range(HPC):
                    nc.tensor.matmul(tp[:, h, :], phiq[:, t, h, :],
                                     id16_sb[:], is_transpose=True)
                phiT = rot.tile([E, HPC, P], BF16, tag="phiT",
                                name=f"phiT_{t}")
                nc.scalar.activation(
                    phiT[:].rearrange("p h n -> p (h n)"),
                    tp[:].rearrange("p h n -> p (h n)"), ACT)
                phiT_sbs[t] = phiT

            def stage_O(t):                      # o = phiT^T @ kvs [P, VW]
                phiT = phiT_sbs.pop(t)
                o_ps = [ps.tile([P, VH], F32, tag="v1", name=f"ops0_{t}"),
                        ps.tile([P, VH], F32, tag="v2", name=f"ops1_{t}")]
                for h in range(HPC):
                    nc.tensor.matmul(
                        o_ps[h // 4][:, (h % 4) * E:(h % 4) * E + E],
                        phiT[:, h, :], kvs_sb[:, h * E:h * E + E],
                        start=(h % 4 == 0), stop=(h % 4 == 3),
                        skip_group_check=True)
                o_sb = rot.tile([P, VW], BF16, tag="osb", name=f"osb_{t}")
                nc.vector.tensor_copy(o_sb[:, 0:VH], o_ps[0][:])
                nc.scalar.activation(o_sb[:, VH:VW], o_ps[1][:], ACT)
                o_sbs[t] = o_sb

            def stage_R(t):                      # o^T -> oT_sb [P, OCH, P]
                o_sb = o_sbs.pop(t)
                tp = ps.tile([P, OCH, P], BF16, tag="kv1", name=f"otp_{t}")
                for c in range(OCH):
                    kk = OLAST if c == OCH - 1 else P
                    nc.tensor.matmul(tp[0:kk, c, :],
                                     o_sb[:, c * P:c * P + kk],
                                     id16_sb[:], is_transpose=True)
                oT = rot.tile([P, OCH, P], BF16, tag="oT", name=f"oT_{t}")
                nc.vector.tensor_copy(
                    oT[:].rearrange("p c n -> p (c n)"),
                    tp[:].rearrange("p c n -> p (c n)"))
                oT_sbs[t] = oT

            def stage_W(t):                      # out = o^T.T @ Wo -> DMA
                oT = oT_sbs.pop(t)
                f1 = ps2.tile([P, 512], F32, tag="qk", name=f"f1_{t}")
                f2 = ps2.tile([P, 512], F32, tag="qk", name=f"f2_{t}")
                f3 = ps.tile([P, D - 1024], F32, tag="kv0", name=f"f3_{t}")
                for c in range(OCH):
                    kk = OLAST if c == OCH - 1 else P
                    nc.tensor.matmul(f1[:], oT[0:kk, c, :],
                                     wo_sb[0:kk, c, 0:512],
                                     start=(c == 0), stop=(c == OCH - 1))
                    nc.tensor.matmul(f2[:], oT[0:kk, c, :],
                                     wo_sb[0:kk, c, 512:1024],
                                     start=(c == 0), stop=(c == OCH - 1))
                    nc.tensor.matmul(f3[:], oT[0:kk, c, :],
                                     wo_sb[0:kk, c, 1024:D],
                                     start=(c == 0), stop=(c == OCH - 1))
                out_sb = rot.tile([P, D], F32, tag="outsb", name=f"outsb_{t}")
                nc.vector.tensor_copy(out_sb[:, 0:512], f1[:])
                nc.scalar.activation(out_sb[:, 512:1024], f2[:], ACT)
                nc.vector.tensor_copy(out_sb[:, 1024:D], f3[:])
                nc.sync.dma_start(out=out[t * P:(t + 1) * P, :], in_=out_sb)

            for b in range(NT + 3):
                with nc.named_scope(f"B{b}"):
                    if b < NT:
                        stage_T(b)
                    if 0 <= b - 1 < NT:
                        stage_O(b - 1)
                    if 0 <= b - 2 < NT:
                        stage_R(b - 2)
                    if 0 <= b - 3 < NT:
                        stage_W(b - 3)

    nc.compile()
    return nc


def _prep_core_inputs(hidden_states, attention_mask, Wq, Wk, Wv, Wo, core):
    b, half = core // 2, core % 2
    h0 = half * HPC
    bf = ml_dtypes.bfloat16

    hs = np.ascontiguousarray(hidden_states[b]).astype(np.float32)
    maskf = np.ascontiguousarray(
        attention_mask[b].astype(np.float32).reshape(NT, P).T)

    def chunks(w):
        out = np.zeros((NCH, P, w.shape[1]), dtype=np.float32)
        for c in range(NCH):
            kk = _chunk_k(c)
            out[c, 0:kk] = w[c * P:c * P + kk]
        return out

    wq_h = Wq[:, h0 * F:(h0 + HPC) * F].astype(np.float32)
    wk_h = Wk[:, h0 * F:(h0 + HPC) * F].astype(np.float32)
    wqk_h = chunks(np.concatenate([wq_h, wk_h], axis=1))
    wv_h = chunks(Wv[:, h0 * E:(h0 + HPC) * E].astype(np.float32)).astype(bf)
    wo_rows = Wo[h0 * E:(h0 + HPC) * E].astype(np.float32)
    wo_h = np.zeros((OCH, P, D), dtype=np.float32)
    for c in range(OCH):
        kk = OLAST if c == OCH - 1 else P
        wo_h[c, 0:kk] = wo_rows[c * P:c * P + kk]
    wo_h = wo_h.astype(bf)

    return {
        "hs": hs,
        "maskf": maskf,
        "wqk": wqk_h,
        "wv": wv_h,
        "wo": wo_h,
        "id32": np.eye(P, dtype=np.float32),
        "id16": np.eye(P, dtype=np.float32).astype(bf),
    }


def kernel(hidden_states, attention_mask, Wq, Wk, Wv, Wo, _trace=False):
    hidden_states = np.asarray(hidden_states)
    attention_mask = np.asarray(attention_mask)
    Wq = np.asarray(Wq); Wk = np.asarray(Wk)
    Wv = np.asarray(Wv); Wo = np.asarray(Wo)

    if "nc" not in _CACHED:
        _CACHED["nc"] = build_bass()
    nc = _CACHED["nc"]

    in_maps = [
        _prep_core_inputs(hidden_states, attention_mask, Wq, Wk, Wv, Wo, c)
        for c in range(8)
    ]
    res = run_bass_kernel_spmd(nc, in_maps, core_ids=list(range(8)),
                               trace=_trace)
    _CACHED["last_result"] = res
    out = np.empty((B, S, D), dtype=np.float32)
    for b in range(B):
        out[b] = res.results[2 * b]["out"] + res.results[2 * b + 1]["out"]
    return out


# revision 11
# speedup vs baseline: 1.5804x; 1.0107x over previous
"""Trainium2 Bass kernel for nn_LinearMultiheadAttention (linear attention with
polynomial feature map phi(x) = [1, x, 0.5 x^2]), sharded over 8 NeuronCores.

Sharding: core c -> batch b = c//2, heads h0 = (c%2)*8 .. h0+8.
Each core computes a partial output (its 8 heads' contribution through Wo);
the host sums the two partials per batch.

Precision: the z = qsum*ksum normalizer is catastrophically ill-conditioned
(min |qsum| ~3e-4 while outputs reach 6e5), so q/k projections are computed
to full fp32 accuracy via an exact 3-term fp32r split
(hs = hi + lo, W = Whi + Wlo, 12+12 mantissa bits, products exact in the
fp32 PSUM accumulate; only the lo*lo term ~2^-24 is dropped). qsum/ksum are
accumulated in exact fp32. The v / kv / qkv / Wo path is bf16.

v2 scheduling: kv matmuls deferred one tile (no PE head-of-line block on the
phi_k build), no gpsimd anywhere (measured ~20x below spec), matmul-based
ksum assembly in mid (no SBUF-SBUF DMAs / transposes), hs prefetched before
the big weight DMAs, pass B software-pipelined 3 deep with single-bank bf16
transpose targets.
"""
import numpy as np
import ml_dtypes

import concourse.bass as bass
import concourse.tile as tile
from concourse import bacc, mybir
from concourse.bass_utils import run_bass_kernel_spmd

F32 = mybir.dt.float32
F32R = mybir.dt.float32r
BF16 = mybir.dt.bfloat16

B, S, D = 4, 4096, 1040
H, F, E = 16, 32, 65          # heads, feature_dim, head_dim (= 2F+1)
HPC = 8                        # heads per core
P = 128
NT = S // P                    # 32 token tiles per core
NCH = 9                        # ceil(D/128); last chunk K=16
KLAST = D - 8 * P              # 16
QW = HPC * F                   # 256 q (or k) cols per core
VW = HPC * E                   # 520 v cols per core
VH = 4 * E                     # 260
OCH = 5                        # ceil(VW/128); last chunk K=8
OLAST = VW - 4 * P             # 8

_CACHED = {}


def _chunk_k(c):
    return KLAST if c == NCH - 1 else P


def build_bass():
    nc = bacc.Bacc("TRN2", target_bir_lowering=False, debug=False, num_devices=8)
    hs = nc.dram_tensor("hs", [S, D], F32, kind="ExternalInput").ap()
    maskf = nc.dram_tensor("maskf", [P, NT], F32, kind="ExternalInput").ap()
    wqk = nc.dram_tensor("wqk", [NCH, P, 2 * QW], F32, kind="ExternalInput").ap()
    wv = nc.dram_tensor("wv", [NCH, P, VW], BF16, kind="ExternalInput").ap()
    wo = nc.dram_tensor("wo", [OCH, P, D], BF16, kind="ExternalInput").ap()
    id32 = nc.dram_tensor("id32", [P, P], F32, kind="ExternalInput").ap()
    id16 = nc.dram_tensor("id16", [P, P], BF16, kind="ExternalInput").ap()
    out = nc.dram_tensor("out", [S, D], F32, kind="ExternalOutput").ap()

    ACT = mybir.ActivationFunctionType.Copy

    with tile.TileContext(nc) as tc:
        with (
            tc.tile_pool(name="consts", bufs=1) as consts,
            tc.tile_pool(name="state", bufs=1) as state,
            tc.tile_pool(name="rot", bufs=2) as rot,
            tc.tile_pool(name="ps", bufs=1, space="PSUM") as ps,
            tc.tile_pool(name="ps2", bufs=2, space="PSUM") as ps2,
        ):
            # ---- small consts + hs prefetch BEFORE the big weight DMAs ----
            id32_sb = consts.tile([P, P], F32)
            nc.sync.dma_start(out=id32_sb, in_=id32)
            hs_tiles = [consts.tile([P, D], F32, name=f"hst{i}")
                        for i in range(3)]
            nc.sync.dma_start(out=hs_tiles[0], in_=hs[0:P, :])
            id16_sb = consts.tile([P, P], BF16)
            nc.sync.dma_start(out=id16_sb, in_=id16)
            mask_sb = consts.tile([P, NT], F32)
            nc.sync.dma_start(out=mask_sb, in_=maskf)
            for i in (1, 2):
                nc.sync.dma_start(out=hs_tiles[i], in_=hs[i * P:(i + 1) * P, :])

            # weights chunk-by-chunk so the first qk matmuls start early
            wqk_sb = consts.tile([P, NCH, 2 * QW], F32)
            wqkr = consts.tile([P, NCH, 2 * QW], F32R)
            wqkl = consts.tile([P, NCH, 2 * QW], F32R)
            for c in range(NCH):
                nc.sync.dma_start(out=wqk_sb[:, c, :],
                                  in_=wqk[c].rearrange("p j -> p j"))
                nc.vector.tensor_copy(wqkr[:, c, :], wqk_sb[:, c, :])
                nc.vector.tensor_tensor(wqkl[:, c, :], wqk_sb[:, c, :],
                                        wqkr[:, c, :].bitcast(F32),
                                        mybir.AluOpType.subtract)
            wv_sb = consts.tile([P, NCH, VW], BF16)
            for c in range(NCH):
                nc.sync.dma_start(out=wv_sb[:, c, :],
                                  in_=wv[c].rearrange("p j -> p j"))
            wo_sb = consts.tile([P, OCH, D], BF16)
            nc.sync.dma_start(out=wo_sb, in_=wo.rearrange("c p j -> p c j"))

            ones_col = consts.tile([P, 1], F32)
            nc.vector.memset(ones_col[:], 1.0)
            half_col = consts.tile([P, 1], F32)
            nc.vector.memset(half_col[:], 0.5)
            ones_row = consts.tile([1, P], F32)
            nc.vector.memset(ones_row[:], 1.0)

            # ---- persistent state ----
            phiq = state.tile([P, NT, HPC, E], BF16)   # rq-folded phi(q) stash
            kvs_sb = state.tile([E, HPC * E], BF16)    # rksum-scaled kv
            rk_row = state.tile([1, HPC * E], F32)
            klin_acc = state.tile([P, QW], F32)
            ksq_acc = state.tile([P, QW], F32)
            kv_ps = [ps.tile([E, VH], F32, tag=f"kv{i}", name=f"kv{i}")
                     for i in range(2)]
            # phik/v16 live one extra tile (kv deferral) -> explicit 2 bufs
            phik_t = [state.tile([P, HPC, E], BF16, name=f"phik{i}")
                      for i in range(2)]
            v16_t = [state.tile([P, VW], BF16, name=f"v16{i}")
                     for i in range(2)]
            # ones column of phi_k never changes: set once per buffer
            for i in range(2):
                nc.vector.memset(phik_t[i][:, :, 0:1], 1.0)

            def kv_mm(t):
                pk, vv = phik_t[t % 2], v16_t[t % 2]
                for h in range(HPC):
                    nc.tensor.matmul(
                        kv_ps[h // 4][:, (h % 4) * E:(h % 4) * E + E],
                        pk[:, h, :], vv[:, h * E:h * E + E],
                        start=(t == 0 and h % 4 == 0), stop=(t == NT - 1),
                        skip_group_check=True)

            # =============== PASS A ===============
            for t in range(NT):
                with nc.named_scope(f"A{t}"):
                    hs_t = hs_tiles[t % 3]

                    hsr = rot.tile([P, NCH, P], F32R, tag="hsr")
                    hlo = rot.tile([P, NCH, P], F32R, tag="hlo")
                    hsT16 = rot.tile([P, NCH, P], BF16, tag="hsT16")
                    for g, cs in enumerate([range(0, 4), range(4, 8), range(8, 9)]):
                        tp = ps2.tile([P, 512], F32, tag="tps",
                                      name=f"tp_{t}_{g}")
                        for c in cs:
                            kk = _chunk_k(c)
                            nc.tensor.transpose(
                                tp[0:kk, (c % 4) * P:(c % 4) * P + P],
                                hs_t[:, c * P:c * P + kk],
                                id32_sb[:])
                        lo, hi = cs[0], cs[-1] + 1
                        kk = _chunk_k(hi - 1)
                        w = (hi - 1 - lo) * P + P
                        src = tp[0:kk, 0:w]
                        hr = hsr[0:kk, lo:hi, :].rearrange("p c n -> p (c n)")
                        nc.scalar.activation(hr, src, ACT)
                        nc.vector.tensor_tensor(
                            hlo[0:kk, lo:hi, :].rearrange("p c n -> p (c n)"),
                            src, hr.bitcast(F32), mybir.AluOpType.subtract)
                        nc.scalar.activation(
                            hsT16[0:kk, lo:hi, :].rearrange("p c n -> p (c n)"),
                            src, ACT)

                    # refill this hs slot for tile t+3 (readers above queued)
                    if t + 3 < NT:
                        nc.sync.dma_start(
                            out=hs_t, in_=hs[(t + 3) * P:(t + 4) * P, :])

                    # projections: q|k via exact 3-term fp32r, v via bf16
                    qk_ps = ps2.tile([P, 2 * QW], F32, tag="qk", name=f"qk_{t}")
                    v1_ps = ps.tile([P, VH], F32, tag="v1", name=f"v1_{t}")
                    v2_ps = ps.tile([P, VH], F32, tag="v2", name=f"v2_{t}")
                    for c in range(NCH):
                        kk = _chunk_k(c)
                        nc.tensor.matmul(
                            qk_ps[:], hsr[0:kk, c, :], wqkr[0:kk, c, :],
                            start=(c == 0), stop=False, skip_group_check=True)
                        nc.tensor.matmul(
                            qk_ps[:], hsr[0:kk, c, :], wqkl[0:kk, c, :],
                            start=False, stop=False, skip_group_check=True)
                    for c in range(NCH):
                        kk = _chunk_k(c)
                        nc.tensor.matmul(
                            qk_ps[:], hlo[0:kk, c, :], wqkr[0:kk, c, :],
                            start=False, stop=(c == NCH - 1),
                            skip_group_check=True)
                    for c in range(NCH):
                        kk = _chunk_k(c)
                        nc.tensor.matmul(
                            v1_ps[:], hsT16[0:kk, c, :], wv_sb[0:kk, c, 0:VH],
                            start=(c == 0), stop=(c == NCH - 1))
                        nc.tensor.matmul(
                            v2_ps[:], hsT16[0:kk, c, :], wv_sb[0:kk, c, VH:VW],
                            start=(c == 0), stop=(c == NCH - 1))

                    # kv for the PREVIOUS tile (its phik/v16 are long ready,
                    # so the PE never blocks on the vector chain below)
                    if t > 0:
                        kv_mm(t - 1)

                    # exact fp32 copies + squares
                    qf32 = rot.tile([P, QW], F32, tag="qf32")
                    nc.scalar.activation(qf32[:], qk_ps[:, 0:QW], ACT)
                    kf32 = rot.tile([P, QW], F32, tag="kf32")
                    nc.scalar.activation(kf32[:], qk_ps[:, QW:2 * QW], ACT)
                    sq2 = rot.tile([P, QW], F32, tag="sq2")
                    nc.vector.tensor_mul(sq2[:], qk_ps[:, 0:QW], qf32[:])
                    sk2 = rot.tile([P, QW], F32, tag="sk2")
                    nc.vector.tensor_mul(sk2[:], qk_ps[:, QW:2 * QW], kf32[:])

                    # ksum accumulators (per-partition partial sums, exact fp32)
                    if t == 0:
                        nc.vector.tensor_copy(klin_acc[:], kf32[:])
                        nc.vector.tensor_copy(ksq_acc[:], sk2[:])
                    else:
                        nc.vector.tensor_add(klin_acc[:], klin_acc[:], kf32[:])
                        nc.vector.tensor_add(ksq_acc[:], ksq_acc[:], sk2[:])

                    # qsum = 1 + sum(q) + 0.5*sum(q^2); rq = mask/qsum
                    sumq = rot.tile([P, HPC], F32, tag="sumq")
                    nc.vector.tensor_reduce(
                        sumq[:], qf32[:].rearrange("p (h f) -> p h f", f=F),
                        mybir.AxisListType.X, mybir.AluOpType.add)
                    sumq2 = rot.tile([P, HPC], F32, tag="sumq2")
                    nc.vector.tensor_reduce(
                        sumq2[:], sq2[:].rearrange("p (h f) -> p h f", f=F),
                        mybir.AxisListType.X, mybir.AluOpType.add)
                    qsum = rot.tile([P, HPC], F32, tag="qsum")
                    nc.vector.tensor_scalar(
                        qsum[:], sumq2[:], 0.5, 1.0,
                        mybir.AluOpType.mult, mybir.AluOpType.add)
                    nc.vector.tensor_add(qsum[:], qsum[:], sumq[:])
                    rq = rot.tile([P, HPC], F32, tag="rq")
                    nc.vector.reciprocal(rq[:], qsum[:])
                    nc.vector.tensor_mul(
                        rq[:], rq[:], mask_sb[:, t:t + 1].broadcast_to([P, HPC]))
                    rq05 = rot.tile([P, HPC], F32, tag="rq05")
                    nc.vector.tensor_scalar_mul(rq05[:], rq[:], 0.5)

                    # phi_q (rq folded) -> stash (bf16)
                    pq = phiq[:, t]                      # [P, HPC, E]
                    nc.vector.tensor_copy(pq[:, :, 0:1], rq[:].unsqueeze(2))
                    nc.vector.tensor_mul(
                        pq[:, :, 1:1 + F],
                        qf32[:].rearrange("p (h f) -> p h f", f=F),
                        rq[:].unsqueeze(2).broadcast_to([P, HPC, F]))
                    nc.vector.tensor_mul(
                        pq[:, :, 1 + F:E],
                        sq2[:].rearrange("p (h f) -> p h f", f=F),
                        rq05[:].unsqueeze(2).broadcast_to([P, HPC, F]))

                    # phi_k (bf16, ones col preset) and v (bf16)
                    pk = phik_t[t % 2]
                    nc.scalar.activation(
                        pk[:, :, 1:1 + F],
                        kf32[:].rearrange("p (h f) -> p h f", f=F), ACT)
                    nc.vector.tensor_scalar_mul(
                        pk[:, :, 1 + F:E],
                        sk2[:].rearrange("p (h f) -> p h f", f=F), 0.5)
                    v16 = v16_t[t % 2]
                    nc.scalar.activation(v16[:, 0:VH], v1_ps[:], ACT)
                    nc.scalar.activation(v16[:, VH:VW], v2_ps[:], ACT)

            kv_mm(NT - 1)

            # =============== MID: ksum assembly (matmul-based) ===============
            with nc.named_scope("mid"):
                # column sums of klin/ksq -> [1, 512] on partition 0
                sums_ps = ps2.tile([1, 512], F32, tag="tps", name="sums_ps")
                nc.tensor.matmul(sums_ps[:, 0:QW], ones_col[:], klin_acc[:],
                                 start=True, stop=False, skip_group_check=True)
                nc.tensor.matmul(sums_ps[:, QW:2 * QW], half_col[:], ksq_acc[:],
                                 start=True, stop=True, skip_group_check=True)

                rk_view = rk_row[:].rearrange("o (h e) -> o h e", e=E)
                nc.vector.memset(rk_view[:, :, 0:1], float(S))
                nc.vector.tensor_copy(
                    rk_view[:, :, 1:1 + F],
                    sums_ps[:, 0:QW].rearrange("o (h f) -> o h f", f=F))
                nc.vector.tensor_copy(
                    rk_view[:, :, 1 + F:E],
                    sums_ps[:, QW:2 * QW].rearrange("o (h f) -> o h f", f=F))
                nc.vector.reciprocal(rk_row[:], rk_row[:])

                # broadcast rk over 65 partitions via PE, then scale kv
                rk_sb = state.tile([E, HPC * E], F32)
                for i in range(2):
                    rk_ps = ps.tile([E, VH], F32, tag=f"v{i + 1}",
                                    name=f"rk_ps{i}")
                    nc.tensor.matmul(rk_ps[:], ones_row[:, 0:E],
                                     rk_row[:, i * VH:(i + 1) * VH],
                                     start=True, stop=True,
                                     skip_group_check=True)
                    nc.scalar.activation(rk_sb[:, i * VH:(i + 1) * VH],
                                         rk_ps[:], ACT)
                    nc.vector.tensor_mul(
                        kvs_sb[:, i * VH:(i + 1) * VH],
                        kv_ps[i][:], rk_sb[:, i * VH:(i + 1) * VH])

            # =============== PASS B (3-deep software pipeline) ===============
            # stages for tile t: T=phiq transpose, O=o matmuls, R=o transpose,
            # W=wo matmuls. body(t) issues T(t+1) O(t) R(t-1) W(t-2).
            phiT_sbs, o_sbs, oT_sbs = {}, {}, {}

            def stage_T(t):                      # phiq -> phiT_sb [E, HPC, P]
                tp = ps2.tile([E, HPC, P], BF16, tag="tps", name=f"ptp_{t}",
                              bufs=2)
                for h in range(HPC):
                    nc.tensor.matmul(tp[:, h, :], phiq[:, t, h, :],
                                     id16_sb[:], is_transpose=True)
                phiT = rot.tile([E, HPC, P], BF16, tag="phiT",
                                name=f"phiT_{t}")
                nc.scalar.activation(
                    phiT[:].rearrange("p h n -> p (h n)"),
                    tp[:].rearrange("p h n -> p (h n)"), ACT)
                phiT_sbs[t] = phiT

            def stage_O(t):                      # o = phiT^T @ kvs [P, VW]
                phiT = phiT_sbs.pop(t)
                o_ps = [ps.tile([P, VH], F32, tag="v1", name=f"ops0_{t}"),
                        ps.tile([P, VH], F32, tag="v2", name=f"ops1_{t}")]
                for h in range(HPC):
                    nc.tensor.matmul(
                        o_ps[h // 4][:, (h % 4) * E:(h % 4) * E + E],
                        phiT[:, h, :], kvs_sb[:, h * E:h * E + E],
                        start=(h % 4 == 0), stop=(h % 4 == 3),
                        skip_group_check=True)
                o_sb = rot.tile([P, VW], BF16, tag="osb", name=f"osb_{t}")
                nc.vector.tensor_copy(o_sb[:, 0:VH], o_ps[0][:])
                nc.scalar.activation(o_sb[:, VH:VW], o_ps[1][:], ACT)
                o_sbs[t] = o_sb

            def stage_R(t):                      # o^T -> oT_sb [P, OCH, P]
                o_sb = o_sbs.pop(t)
                tp = ps.tile([P, OCH, P], BF16, tag="kv1", name=f"otp_{t}")
                for c in range(OCH):
                    kk = OLAST if c == OCH - 1 else P
                    nc.tensor.matmul(tp[0:kk, c, :],
                                     o_sb[:, c * P:c * P + kk],
                                     id16_sb[:], is_transpose=True)
                oT = rot.tile([P, OCH, P], BF16, tag="oT", name=f"oT_{t}")
                nc.vector.tensor_copy(
                    oT[:].rearrange("p c n -> p (c n)"),
                    tp[:].rearrange("p c n -> p (c n)"))
                oT_sbs[t] = oT

            def stage_W(t):                      # out = o^T.T @ Wo -> DMA
                oT = oT_sbs.pop(t)
                f1 = ps2.tile([P, 512], F32, tag="qk", name=f"f1_{t}")
                f2 = ps2.tile([P, 512], F32, tag="qk", name=f"f2_{t}")
                f3 = ps.tile([P, D - 1024], F32, tag="kv0", name=f"f3_{t}")
                for c in range(OCH):
                    kk = OLAST if c == OCH - 1 else P
                    nc.tensor.matmul(f1[:], oT[0:kk, c, :],
                                     wo_sb[0:kk, c, 0:512],
                                     start=(c == 0), stop=(c == OCH - 1))
                    nc.tensor.matmul(f2[:], oT[0:kk, c, :],
                                     wo_sb[0:kk, c, 512:1024],
                                     start=(c == 0), stop=(c == OCH - 1))
                    nc.tensor.matmul(f3[:], oT[0:kk, c, :],
                                     wo_sb[0:kk, c, 1024:D],
                                     start=(c == 0), stop=(c == OCH - 1))
                out_sb = rot.tile([P, D], F32, tag="outsb", name=f"outsb_{t}")
                nc.vector.tensor_copy(out_sb[:, 0:512], f1[:])
                nc.scalar.activation(out_sb[:, 512:1024], f2[:], ACT)
                nc.vector.tensor_copy(out_sb[:, 1024:D], f3[:])
                nc.sync.dma_start(out=out[t * P:(t + 1) * P, :], in_=out_sb)

            for b in range(NT + 3):
                with nc.named_scope(f"B{b}"):
                    if b < NT:
                        stage_T(b)
                    if 0 <= b - 1 < NT:
                        stage_O(b - 1)
                    if 0 <= b - 2 < NT:
                        stage_R(b - 2)
                    if 0 <= b - 3 < NT:
                        stage_W(b - 3)

    nc.compile()
    return nc


def _prep_core_inputs(hidden_states, attention_mask, Wq, Wk, Wv, Wo, core):
    b, half = core // 2, core % 2
    h0 = half * HPC
    bf = ml_dtypes.bfloat16

    hs = np.ascontiguousarray(hidden_states[b]).astype(np.float32)
    maskf = np.ascontiguousarray(
        attention_mask[b].astype(np.float32).reshape(NT, P).T)

    def chunks(w):
        out = np.zeros((NCH, P, w.shape[1]), dtype=np.float32)
        for c in range(NCH):
            kk = _chunk_k(c)
            out[c, 0:kk] = w[c * P:c * P + kk]
        return out

    wq_h = Wq[:, h0 * F:(h0 + HPC) * F].astype(np.float32)
    wk_h = Wk[:, h0 * F:(h0 + HPC) * F].astype(np.float32)
    wqk_h = chunks(np.concatenate([wq_h, wk_h], axis=1))
    wv_h = chunks(Wv[:, h0 * E:(h0 + HPC) * E].astype(np.float32)).astype(bf)
    wo_rows = Wo[h0 * E:(h0 + HPC) * E].astype(np.float32)
    wo_h = np.zeros((OCH, P, D), dtype=np.float32)
    for c in range(OCH):
        kk = OLAST if c == OCH - 1 else P
        wo_h[c, 0:kk] = wo_rows[c * P:c * P + kk]
    wo_h = wo_h.astype(bf)

    return {
        "hs": hs,
        "maskf": maskf,
        "wqk": wqk_h,
        "wv": wv_h,
        "wo": wo_h,
        "id32": np.eye(P, dtype=np.float32),
        "id16": np.eye(P, dtype=np.float32).astype(bf),
    }


def kernel(hidden_states, attention_mask, Wq, Wk, Wv, Wo, _trace=False):
    hidden_states = np.asarray(hidden_states)
    attention_mask = np.asarray(attention_mask)
    Wq = np.asarray(Wq); Wk = np.asarray(Wk)
    Wv = np.asarray(Wv); Wo = np.asarray(Wo)

    if "nc" not in _CACHED:
        _CACHED["nc"] = build_bass()
    nc = _CACHED["nc"]

    in_maps = [
        _prep_core_inputs(hidden_states, attention_mask, Wq, Wk, Wv, Wo, c)
        for c in range(8)
    ]
    res = run_bass_kernel_spmd(nc, in_maps, core_ids=list(range(8)),
                               trace=_trace)
    _CACHED["last_result"] = res
    out = np.empty((B, S, D), dtype=np.float32)
    for b in range(B):
        out[b] = res.results[2 * b]["out"] + res.results[2 * b + 1]["out"]
    return out
